# revision 5
# baseline (speedup 1.0000x reference)
"""Trainium2 Bass kernel for nn_DQN_34136400069239 (DeepSets-style pooling).

Math (reference):
    h1  = relu(x @ pw1 + pb1)          [N, H]
    h2  = relu(h1 @ pw2 + pb2)         [N, H]
    phi = h2 @ pw3 + pb3               [N, F]
    fp  = sum(phi, axis=0)             [F]
    ... tiny rho MLP + concat(x_static) + tiny 3-layer MLP -> [OUT]

Key algebraic simplification: the third phi layer is linear, so
    fp = (sum_n h2[n]) @ pw3 + N * pb3
and the device kernel only needs S = sum_n relu(h1[n] @ pw2 + pb2) in R^H.
The rho network + final MLP operate on a single vector and run on host.

Device strategy (per the data-parallel sharding hint):
  - shard x rows across 8 NeuronCores (50000 rows each)
  - per core, stream xT blocks [65, 500] (row 64 = ones so pb1 rides in the
    matmul contraction), two-layer MLP on the PE in fp32r (full-rate
    reduced-precision fp32), fused relu(+bias)+row-sum on ACT/DVE engines
  - each core outputs its partial S [H]; host sums the 8 partials and runs
    the tiny tail in float64.
"""

import os

import numpy as np

# Problem constants (hardcoded; kernel.py must be self-contained).
N = 400000
IN, H, F, S_STATIC, OUT = 64, 256, 128, 16, 5
N_CORES = 8
R = N // N_CORES  # rows per core = 50000
BLK = 500  # matmul moving free dim (<=512 for 4-byte dtypes)
NBLK = R // BLK  # 100

# Precision mode: "f32r" | "f32r_split" | "f32"
MODE = os.environ.get("DQN_MODE", "f32r_split")

_prog_cache: dict = {}


def _build(mode: str):
    import concourse.mybir as mybir
    import concourse.tile as tile
    from concourse import bacc
    from contextlib import ExitStack

    dt = mybir.dt
    f32 = dt.float32
    split = mode == "f32r_split"
    mm_dt = {"f32r": dt.float32r, "f32r_split": dt.float32r, "f32": f32}[mode]

    nc = bacc.Bacc(
        "TRN2",
        target_bir_lowering=False,
        debug=False,
        enable_asserts=False,
        num_devices=1,
    )

    d_xt = nc.dram_tensor("d_xt", [IN + 1, R], mm_dt, kind="ExternalInput").ap()
    d_w1 = nc.dram_tensor("d_w1", [IN + 1, H], mm_dt, kind="ExternalInput").ap()
    d_w2 = nc.dram_tensor("d_w2", [H, H], mm_dt, kind="ExternalInput").ap()
    d_pb2 = nc.dram_tensor("d_pb2", [H], f32, kind="ExternalInput").ap()
    if split:
        d_w1l = nc.dram_tensor("d_w1l", [IN + 1, H], mm_dt, kind="ExternalInput").ap()
        d_w2l = nc.dram_tensor("d_w2l", [H, H], mm_dt, kind="ExternalInput").ap()
    d_s = nc.dram_tensor("d_s", [128, 2], f32, kind="ExternalOutput").ap()

    Relu = mybir.ActivationFunctionType.Relu
    X = mybir.AxisListType.X

    with tile.TileContext(nc) as tc, ExitStack() as ctx:
        cpool = ctx.enter_context(tc.tile_pool(name="cpool", bufs=1))
        xpool = ctx.enter_context(tc.tile_pool(name="xpool", bufs=4))
        hpool = ctx.enter_context(tc.tile_pool(name="hpool", bufs=3))
        spool = ctx.enter_context(tc.tile_pool(name="spool", bufs=2))
        ps1p = ctx.enter_context(tc.tile_pool(name="ps1p", bufs=2, space="PSUM"))
        ps2p = ctx.enter_context(tc.tile_pool(name="ps2p", bufs=2, space="PSUM"))

        # Weights / biases resident in SBUF.
        w1_sb = cpool.tile([IN + 1, H], mm_dt, name="w1_sb")
        nc.sync.dma_start(w1_sb[:], d_w1)
        w2_sb = []
        for k in range(2):
            t = cpool.tile([128, H], mm_dt, name=f"w2_sb{k}")
            nc.sync.dma_start(t[:], d_w2[k * 128 : (k + 1) * 128, :])
            w2_sb.append(t)
        if split:
            w1l_sb = cpool.tile([IN + 1, H], mm_dt, name="w1l_sb")
            nc.sync.dma_start(w1l_sb[:], d_w1l)
            w2l_sb = []
            for k in range(2):
                t = cpool.tile([128, H], mm_dt, name=f"w2l_sb{k}")
                nc.sync.dma_start(t[:], d_w2l[k * 128 : (k + 1) * 128, :])
                w2l_sb.append(t)
        pb2_sb = cpool.tile([128, 2], f32, name="pb2_sb")
        nc.sync.dma_start(pb2_sb[:], d_pb2.rearrange("(m p) -> p m", p=128))

        # Per-block partial sums of relu(h2): [128 partitions, 2 halves, NBLK].
        acc = cpool.tile([128, 2, NBLK], f32, name="acc")

        for b in range(NBLK):
            xt = xpool.tile([IN + 1, BLK], mm_dt, name="xt", tag="xt")
            nc.sync.dma_start(xt[:], d_xt[:, b * BLK : (b + 1) * BLK])
            xr = xt[:]

            # Layer 1: h1T[m*128+p, col] in PSUM, bias via the ones row.
            ps1 = ps1p.tile([128, 2, 512], f32, name="ps1", tag="ps1")
            for m in range(2):
                ms = slice(m * 128, (m + 1) * 128)
                nc.tensor.matmul(
                    ps1[:, m, 0:BLK],
                    w1_sb[:, ms],
                    xr,
                    start=True,
                    stop=not split,
                )
                if split:
                    nc.tensor.matmul(
                        ps1[:, m, 0:BLK],
                        w1l_sb[:, ms],
                        xr,
                        start=False,
                        stop=True,
                    )

            # relu over both halves in one DVE op.
            h1 = hpool.tile([128, 2, BLK], mm_dt, name="h1", tag="h1")
            nc.vector.tensor_scalar_max(h1[:], ps1[:, :, 0:BLK], 0.0)

            # Layer 2: accumulate over the two K chunks.
            ps2 = ps2p.tile([128, 2, 512], f32, name="ps2", tag="ps2")
            for m in range(2):
                ms = slice(m * 128, (m + 1) * 128)
                mms = []
                for k in range(2):
                    mms.append((w2_sb[k][:, ms], h1[:, k, :]))
                    if split:
                        mms.append((w2l_sb[k][:, ms], h1[:, k, :]))
                for i, (lw, rr) in enumerate(mms):
                    nc.tensor.matmul(
                        ps2[:, m, 0:BLK],
                        lw,
                        rr,
                        start=(i == 0),
                        stop=(i == len(mms) - 1),
                    )

            # relu(h2 + pb2) with fused row-sum into acc column b (ACT engine).
            for m in range(2):
                scr = spool.tile([128, BLK], f32, name=f"scr{m}", tag=f"scr{m}")
                nc.scalar.activation(
                    scr[:],
                    ps2[:, m, 0:BLK],
                    Relu,
                    bias=pb2_sb[:, m : m + 1],
                    accum_out=acc[:, m, b : b + 1],
                )

        # Reduce per-block sums -> [128, 2] and store.
        s_sb = cpool.tile([128, 2], f32, name="s_sb")
        nc.vector.reduce_sum(s_sb[:], acc[:], axis=X)
        nc.sync.dma_start(d_s, s_sb[:])

    nc.compile()
    return nc


def _hi_lo(w: np.ndarray):
    import ml_dtypes

    hi = np.asarray(w, dtype=ml_dtypes.bfloat16).astype(np.float32)
    lo = (w - hi).astype(np.float32)
    return hi, lo


def _prep_in_maps(inputs: dict, mode: str):
    split = mode == "f32r_split"
    x = np.asarray(inputs["x"], dtype=np.float32)
    pw1 = np.asarray(inputs["pw1"], dtype=np.float32)
    pb1 = np.asarray(inputs["pb1"], dtype=np.float32)
    pw2 = np.asarray(inputs["pw2"], dtype=np.float32)
    pb2 = np.asarray(inputs["pb2"], dtype=np.float32)

    w1_aug = np.concatenate([pw1, pb1[None, :]], axis=0)  # [65, H]
    if split:
        w1h, w1l = _hi_lo(w1_aug)
        w2h, w2l = _hi_lo(pw2)
    else:
        w1h, w2h = w1_aug, pw2

    in_maps = []
    for c in range(N_CORES):
        xt = np.empty((IN + 1, R), np.float32)
        xt[:IN] = x[c * R : (c + 1) * R].T
        xt[IN] = 1.0
        m = {"d_xt": xt, "d_w1": w1h, "d_w2": w2h, "d_pb2": pb2}
        if split:
            m["d_w1l"] = w1l
            m["d_w2l"] = w2l
        in_maps.append(m)
    return in_maps


def _host_tail(S: np.ndarray, inputs: dict) -> np.ndarray:
    f = np.float64

    def g(name):
        return np.asarray(inputs[name], dtype=f)

    phi_sum = S @ g("pw3") + N * g("pb3")
    r = np.maximum(phi_sum @ g("rw1") + g("rb1"), 0.0)
    r = np.maximum(r @ g("rw2") + g("rb2"), 0.0)
    r = r @ g("rw3") + g("rb3")
    v = np.concatenate([r, g("x_static")])
    v = np.maximum(v @ g("w1") + g("b1"), 0.0)
    v = np.maximum(v @ g("w2") + g("b2"), 0.0)
    return (v @ g("w3") + g("b3")).astype(np.float32)


def _run(inputs: dict, trace: bool = False, mode: str | None = None):
    from concourse.bass_utils import run_bass_kernel_spmd

    mode = mode or MODE
    nc = _prog_cache.get(mode)
    if nc is None:
        nc = _build(mode)
        _prog_cache[mode] = nc

    if trace:
        try:
            import antenv.axon_hooks  # noqa: F401
        except ImportError:
            trace = False  # NTFF profiling hook unavailable in this env

    in_maps = _prep_in_maps(inputs, mode)
    res = run_bass_kernel_spmd(
        nc,
        in_maps,
        core_ids=list(range(N_CORES)),
        trace=trace,
    )

    S = np.zeros(H, np.float64)
    for rmap in res.results:
        S += rmap["d_s"].astype(np.float64).T.reshape(H)
    out = _host_tail(S, inputs)
    return out, res


def kernel(**inputs) -> np.ndarray:
    out, _ = _run(inputs)
    return out


# revision 6
# speedup vs baseline: 641.3986x; 641.3986x over previous
"""Trainium2 Bass kernel for nn_DQN_34136400069239 (DeepSets-style pooling).

Math (reference):
    h1  = relu(x @ pw1 + pb1)          [N, H]
    h2  = relu(h1 @ pw2 + pb2)         [N, H]
    phi = h2 @ pw3 + pb3               [N, F]
    fp  = sum(phi, axis=0)             [F]
    ... tiny rho MLP + concat(x_static) + tiny 3-layer MLP -> [OUT]

Key algebraic simplification: the third phi layer is linear, so
    fp = (sum_n h2[n]) @ pw3 + N * pb3
and the device kernel only needs S = sum_n relu(h1[n] @ pw2 + pb2) in R^H.
The rho network + final MLP operate on a single vector and run on host.

Device strategy (per the data-parallel sharding hint):
  - shard x rows across 8 NeuronCores (50000 rows each)
  - per core, stream xT blocks [65, 500] (row 64 = ones so pb1 rides in the
    matmul contraction), two-layer MLP on the PE in fp32r (full-rate
    reduced-precision fp32), fused relu(+bias)+row-sum on ACT/DVE engines
  - each core outputs its partial S [H]; host sums the 8 partials and runs
    the tiny tail in float64.
"""

import os

import numpy as np

# Problem constants (hardcoded; kernel.py must be self-contained).
N = 400000
IN, H, F, S_STATIC, OUT = 64, 256, 128, 16, 5
N_CORES = 8
R = N // N_CORES  # rows per core = 50000
BLK = 500  # matmul moving free dim (<=512 for 4-byte dtypes)
NBLK = R // BLK  # 100

# Precision mode: "f32r" | "f32r_split" | "f32"
MODE = os.environ.get("DQN_MODE", "f32r_split")

_prog_cache: dict = {}


def _build(mode: str, iters: int = 1):
    import concourse.mybir as mybir
    import concourse.tile as tile
    from concourse import bacc
    from contextlib import ExitStack

    dt = mybir.dt
    f32 = dt.float32
    split = mode == "f32r_split"
    mm_dt = {"f32r": dt.float32r, "f32r_split": dt.float32r, "f32": f32}[mode]

    nc = bacc.Bacc(
        "TRN2",
        target_bir_lowering=False,
        debug=False,
        enable_asserts=False,
        num_devices=1,
    )

    d_xt = nc.dram_tensor("d_xt", [IN + 1, R], mm_dt, kind="ExternalInput").ap()
    d_w1 = nc.dram_tensor("d_w1", [IN + 1, H], mm_dt, kind="ExternalInput").ap()
    d_w2 = nc.dram_tensor("d_w2", [H, H], mm_dt, kind="ExternalInput").ap()
    d_pb2 = nc.dram_tensor("d_pb2", [H], f32, kind="ExternalInput").ap()
    if split:
        d_w1l = nc.dram_tensor("d_w1l", [IN + 1, H], mm_dt, kind="ExternalInput").ap()
        d_w2l = nc.dram_tensor("d_w2l", [H, H], mm_dt, kind="ExternalInput").ap()
    d_s = nc.dram_tensor("d_s", [128, 2], f32, kind="ExternalOutput").ap()

    Relu = mybir.ActivationFunctionType.Relu
    X = mybir.AxisListType.X

    with tile.TileContext(nc) as tc, ExitStack() as ctx:
        cpool = ctx.enter_context(tc.tile_pool(name="cpool", bufs=1))
        xpool = ctx.enter_context(tc.tile_pool(name="xpool", bufs=4))
        hpool = ctx.enter_context(tc.tile_pool(name="hpool", bufs=3))
        spool = ctx.enter_context(tc.tile_pool(name="spool", bufs=2))
        ps1p = ctx.enter_context(tc.tile_pool(name="ps1p", bufs=2, space="PSUM"))
        ps2p = ctx.enter_context(tc.tile_pool(name="ps2p", bufs=2, space="PSUM"))

        # Weights / biases resident in SBUF.
        w1_sb = cpool.tile([IN + 1, H], mm_dt, name="w1_sb")
        nc.sync.dma_start(w1_sb[:], d_w1)
        w2_sb = []
        for k in range(2):
            t = cpool.tile([128, H], mm_dt, name=f"w2_sb{k}")
            nc.sync.dma_start(t[:], d_w2[k * 128 : (k + 1) * 128, :])
            w2_sb.append(t)
        if split:
            w1l_sb = cpool.tile([IN + 1, H], mm_dt, name="w1l_sb")
            nc.sync.dma_start(w1l_sb[:], d_w1l)
            w2l_sb = []
            for k in range(2):
                t = cpool.tile([128, H], mm_dt, name=f"w2l_sb{k}")
                nc.sync.dma_start(t[:], d_w2l[k * 128 : (k + 1) * 128, :])
                w2l_sb.append(t)
        pb2_sb = cpool.tile([128, 2], f32, name="pb2_sb")
        nc.sync.dma_start(pb2_sb[:], d_pb2.rearrange("(m p) -> p m", p=128))

        # Per-block partial sums of relu(h2): [128 partitions, 2 halves, NBLK].
        acc = cpool.tile([128, 2, NBLK], f32, name="acc")

        # iters > 1 repeats the whole pass over the same data; used only for
        # slope-based device timing (axon RPC floor hides single-pass time).
        for b in [b for _ in range(iters) for b in range(NBLK)]:
            xt = xpool.tile([IN + 1, BLK], mm_dt, name="xt", tag="xt")
            nc.sync.dma_start(xt[:], d_xt[:, b * BLK : (b + 1) * BLK])
            xr = xt[:]

            # Layer 1: h1T[m*128+p, col] in PSUM, bias via the ones row.
            ps1 = ps1p.tile([128, 2, 512], f32, name="ps1", tag="ps1")
            for m in range(2):
                ms = slice(m * 128, (m + 1) * 128)
                nc.tensor.matmul(
                    ps1[:, m, 0:BLK],
                    w1_sb[:, ms],
                    xr,
                    start=True,
                    stop=not split,
                )
                if split:
                    nc.tensor.matmul(
                        ps1[:, m, 0:BLK],
                        w1l_sb[:, ms],
                        xr,
                        start=False,
                        stop=True,
                    )

            # relu over both halves in one DVE op.
            h1 = hpool.tile([128, 2, BLK], mm_dt, name="h1", tag="h1")
            nc.vector.tensor_scalar_max(h1[:], ps1[:, :, 0:BLK], 0.0)

            # Layer 2: accumulate over the two K chunks.
            ps2 = ps2p.tile([128, 2, 512], f32, name="ps2", tag="ps2")
            for m in range(2):
                ms = slice(m * 128, (m + 1) * 128)
                mms = []
                for k in range(2):
                    mms.append((w2_sb[k][:, ms], h1[:, k, :]))
                    if split:
                        mms.append((w2l_sb[k][:, ms], h1[:, k, :]))
                for i, (lw, rr) in enumerate(mms):
                    nc.tensor.matmul(
                        ps2[:, m, 0:BLK],
                        lw,
                        rr,
                        start=(i == 0),
                        stop=(i == len(mms) - 1),
                    )

            # relu(h2 + pb2) with fused row-sum into acc column b (ACT engine).
            for m in range(2):
                scr = spool.tile([128, BLK], f32, name=f"scr{m}", tag=f"scr{m}")
                nc.scalar.activation(
                    scr[:],
                    ps2[:, m, 0:BLK],
                    Relu,
                    bias=pb2_sb[:, m : m + 1],
                    accum_out=acc[:, m, b : b + 1],
                )

        # Reduce per-block sums -> [128, 2] and store.
        s_sb = cpool.tile([128, 2], f32, name="s_sb")
        nc.vector.reduce_sum(s_sb[:], acc[:], axis=X)
        nc.sync.dma_start(d_s, s_sb[:])

    nc.compile()
    return nc


def _hi_lo(w: np.ndarray):
    import ml_dtypes

    hi = np.asarray(w, dtype=ml_dtypes.bfloat16).astype(np.float32)
    lo = (w - hi).astype(np.float32)
    return hi, lo


def _prep_in_maps(inputs: dict, mode: str):
    split = mode == "f32r_split"
    x = np.asarray(inputs["x"], dtype=np.float32)
    pw1 = np.asarray(inputs["pw1"], dtype=np.float32)
    pb1 = np.asarray(inputs["pb1"], dtype=np.float32)
    pw2 = np.asarray(inputs["pw2"], dtype=np.float32)
    pb2 = np.asarray(inputs["pb2"], dtype=np.float32)

    w1_aug = np.concatenate([pw1, pb1[None, :]], axis=0)  # [65, H]
    if split:
        w1h, w1l = _hi_lo(w1_aug)
        w2h, w2l = _hi_lo(pw2)
    else:
        w1h, w2h = w1_aug, pw2

    in_maps = []
    for c in range(N_CORES):
        xt = np.empty((IN + 1, R), np.float32)
        xt[:IN] = x[c * R : (c + 1) * R].T
        xt[IN] = 1.0
        m = {"d_xt": xt, "d_w1": w1h, "d_w2": w2h, "d_pb2": pb2}
        if split:
            m["d_w1l"] = w1l
            m["d_w2l"] = w2l
        in_maps.append(m)
    return in_maps


def _host_tail(S: np.ndarray, inputs: dict) -> np.ndarray:
    f = np.float64

    def g(name):
        return np.asarray(inputs[name], dtype=f)

    phi_sum = S @ g("pw3") + N * g("pb3")
    r = np.maximum(phi_sum @ g("rw1") + g("rb1"), 0.0)
    r = np.maximum(r @ g("rw2") + g("rb2"), 0.0)
    r = r @ g("rw3") + g("rb3")
    v = np.concatenate([r, g("x_static")])
    v = np.maximum(v @ g("w1") + g("b1"), 0.0)
    v = np.maximum(v @ g("w2") + g("b2"), 0.0)
    return (v @ g("w3") + g("b3")).astype(np.float32)


def _run(inputs: dict, trace: bool = False, mode: str | None = None):
    from concourse.bass_utils import run_bass_kernel_spmd

    mode = mode or MODE
    nc = _prog_cache.get(mode)
    if nc is None:
        nc = _build(mode)
        _prog_cache[mode] = nc

    if trace:
        try:
            import antenv.axon_hooks  # noqa: F401
        except ImportError:
            trace = False  # NTFF profiling hook unavailable in this env

    in_maps = _prep_in_maps(inputs, mode)
    res = run_bass_kernel_spmd(
        nc,
        in_maps,
        core_ids=list(range(N_CORES)),
        trace=trace,
    )

    S = np.zeros(H, np.float64)
    for rmap in res.results:
        S += rmap["d_s"].astype(np.float64).T.reshape(H)
    out = _host_tail(S, inputs)
    return out, res


def kernel(**inputs) -> np.ndarray:
    out, _ = _run(inputs)
    return out
